# revision 1
# baseline (speedup 1.0000x reference)
"""Trainium2 Bass kernel for nn_ASIC_44186623541335.

Soft-logic-gate cellular automaton: 8 layers over a 16384-cell ring; each cell
update is a 32-combo soft-boolean match over its 5-neighborhood, weighted by
sigmoid gates and clipped to [0,1].

Algorithm: the per-cell update  sum_c tw[c,n] * prod_i mu(bit_ci, v_i)  is the
multilinear extension of the gate vector tw[:,n] at the 5 neighbor values.
Per layer the gate vector is first Moebius-transformed (a fixed 32x32 +/-1
matrix, done on the TensorEngine) into monomial coefficients A_S[n]; the
evaluation is then a 31-node Horner tree -- each node one multiply + one add,
no subtracts -- batched per level into 2 vector instructions.

Sharding: grid axis split across 8 cores (2048 cells each + 16-cell shrinking
halo, no inter-core communication). Per core the slice lives as 128 partitions
x 17 cells, free dim = (cell, batch) = 21*32 with a 2-cell halo per partition
refreshed between layers by two SBUF-SBUF DMAs; the outermost halo decays into
the redundant 16-cell margin.
"""

import numpy as np

import concourse.bacc as bacc
import concourse.mybir as mybir
from concourse import bass_utils
from concourse.tile import TileContext

GRID = 16384
LAYERS = 8
NPOS = 32
BATCH = 32
M = 4096
STRIDE = 4
NCORES = 8
CORE_N = GRID // NCORES  # 2048

P = 128          # partitions (grid chunks per core)
W = 17           # output cells per partition
WP = W + 4       # stored cells per partition (2-cell halo each side)
HALO = 2 * LAYERS  # 16: core-level shrinking margin
SPAN_COLS = P * W  # 2176 cells covered per core

F32 = mybir.dt.float32
ALU = mybir.AluOpType
ACT_SIGMOID = mybir.ActivationFunctionType.Sigmoid

# batch columns handled by GPSIMD (rest on VectorE); 0 disables the split
GP_B = 0
# split each tree instruction into j-chunks / b-chunks of this size
# (None = unsplit); smaller chunks trade instruction count for less
# per-op DVE pipeline-drain exposure
CHUNK_J = None
CHUNK_B = None
# number of independent VectorE tree chains (b-range split); >1 interleaves
# chains so dependency latency of one hides under the other's execution
V_SPLIT = 1
# coefficient-pipeline double-buffer depth
COEF_BUFS = 2

_CACHE = {}


def _moebius_lhsT():
    """lhsT[c, s] = T[s, c]; A_s = sum_c T[s,c] * tw_c  (multilinear coeffs).

    T[s, c] = (-1)^popcount(s & ~c) if (c & ~s) == 0 else 0.
    """
    t = np.zeros((NPOS, NPOS), dtype=np.float32)
    for s in range(NPOS):
        for c in range(NPOS):
            if c & ~s & 31:
                continue
            t[s, c] = (-1.0) ** bin(s & ~c & 31).count("1")
    return np.ascontiguousarray(t.T)


def _build_program(nrep=None):
    """nrep: timing-only variant — wraps the 8-layer body in a hardware
    loop executed nrep times (results are then meaningless; used to
    measure per-layer cost from wall-clock deltas)."""
    import contextlib

    nc = bacc.Bacc("TRN2", target_bir_lowering=False, debug=False)
    s0_d = nc.dram_tensor("s0", [P, WP * BATCH], F32, kind="ExternalInput")
    twc_d = nc.dram_tensor(
        "twc", [LAYERS, NPOS, SPAN_COLS], F32, kind="ExternalInput"
    )
    tm_d = nc.dram_tensor("tmat", [NPOS, NPOS], F32, kind="ExternalInput")
    out_d = nc.dram_tensor("out", [P, W * BATCH], F32, kind="ExternalOutput")

    FB = W * BATCH  # 544: free size of one (cell, batch) plane

    with TileContext(nc) as tc:
        with (
            tc.tile_pool(name="state", bufs=1) as sp,
            tc.tile_pool(name="coef", bufs=COEF_BUFS) as cp,
            tc.tile_pool(name="psum", bufs=1, space="PSUM") as pp,
            tc.tile_pool(name="dram", bufs=COEF_BUFS, space="DRAM") as dp,
        ):
            s_a = sp.tile([P, WP * BATCH], F32, tag="s_a")
            s_b = sp.tile([P, WP * BATCH], F32, tag="s_b")
            h1 = sp.tile([P, 16 * FB], F32, tag="h1")
            h2 = sp.tile([P, 8 * FB], F32, tag="h2")
            h3 = sp.tile([P, 4 * FB], F32, tag="h3")
            h4 = sp.tile([P, 2 * FB], F32, tag="h4")
            h5 = sp.tile([P, FB], F32, tag="h5")
            tmat = sp.tile([NPOS, NPOS], F32, tag="tmat")
            asb = sp.tile([NPOS, SPAN_COLS], F32, tag="asb")

            # edge-partition halo columns are never DMA-refreshed; zero-fill
            # once so reads stay finite (values only feed the discarded margin)
            nc.gpsimd.memset(s_a[:], 0.0)
            nc.gpsimd.memset(s_b[:], 0.0)

            nc.sync.dma_start(out=s_a[:], in_=s0_d.ap())
            nc.sync.dma_start(out=tmat[:], in_=tm_d.ap())

            def sview(s, d):
                # state view shifted by d cells: (P, BATCH, W), b-major layout
                return s[:].rearrange("p (b w) -> p b w", w=WP)[
                    :, :, 2 + d : 2 + d + W
                ]

            loop_cm = tc.For_i(0, nrep, 1) if nrep else contextlib.nullcontext()
            with loop_cm:
              for layer in range(LAYERS):
                s_in = s_a if layer % 2 == 0 else s_b
                s_out = s_b if layer % 2 == 0 else s_a

                twr = cp.tile([NPOS, SPAN_COLS], F32, tag="twr")
                tws = cp.tile([NPOS, SPAN_COLS], F32, tag="tws")
                ac = cp.tile([P, NPOS * W], F32, tag="ac")
                ps = pp.tile([NPOS, SPAN_COLS], F32, tag="ps")

                nc.sync.dma_start(out=twr[:], in_=twc_d.ap()[layer])
                nc.scalar.activation(tws[:], twr[:], ACT_SIGMOID)

                # Moebius transform on the PE: A = T @ sigmoid(gates)
                for t0 in range(0, SPAN_COLS, 512):
                    t1 = min(t0 + 512, SPAN_COLS)
                    nc.tensor.matmul(
                        ps[:, t0:t1], tmat[:], tws[:, t0:t1], start=True, stop=True
                    )

                # PSUM -> SBUF (ScalarE sits next to PSUM; ACT is idle here),
                # then re-layout to the chunked (p, sigma, u) tiling via a
                # DRAM bounce (a cross-partition gather is not SBUF->SBUF
                # expressible: both APs would need their partition dim first)
                nc.scalar.copy(asb[:], ps[:])
                adram = dp.tile([NPOS, SPAN_COLS], F32, tag="adram")
                nc.sync.dma_start(out=adram[:], in_=asb[:])
                nc.sync.dma_start(
                    out=ac[:].rearrange("p (s u) -> p s u", u=W),
                    in_=adram[:].rearrange("s (p u) -> p s u", u=W),
                )

                # Horner tree: level k pairs adjacent entries with neighbor
                # value v (shift +2,+1,0,-1,-2):  g[j] = even[j] + v * odd[j]
                # Optionally split along batch columns: VectorE gets b<[0,b_s),
                # GPSIMD independently runs the same tree on b in [b_s,32).
                ac3 = ac[:].rearrange("p (j two u) -> p two j u", two=2, u=W)
                levels = [(h1, h2, 8, 1), (h2, h3, 4, 0), (h3, h4, 2, -1)]

                def svw(s, d, b0, b1):
                    return sview(s, d)[:, b0:b1]

                def hview(hs, cnt, b0, b1):
                    r = hs[:].rearrange(
                        "p (j two b u) -> p two j b u", two=2, b=BATCH, u=W
                    )
                    return r[:, 0][:, :, b0:b1], r[:, 1][:, :, b0:b1]

                def tt4(eng, out, in0, in1, op):
                    cj = CHUNK_J or out.shape[1]
                    cb = CHUNK_B or out.shape[2]
                    for j0 in range(0, out.shape[1], cj):
                        for c0 in range(0, out.shape[2], cb):
                            sl = (
                                slice(None),
                                slice(j0, min(j0 + cj, out.shape[1])),
                                slice(c0, min(c0 + cb, out.shape[2])),
                            )
                            eng.tensor_tensor(out[sl], in0[sl], in1[sl], op)

                def tree(eng, b0, b1):
                    bb = b1 - b0
                    v4 = svw(s_in, 2, b0, b1)
                    h1_4 = h1[:].rearrange("p (j b u) -> p j b u", b=BATCH, u=W)[
                        :, :, b0:b1
                    ]
                    tt4(
                        eng,
                        h1_4,
                        ac3[:, 1][:, :, None].broadcast_to([P, 16, bb, W]),
                        v4[:, None].broadcast_to([P, 16, bb, W]),
                        ALU.mult,
                    )
                    tt4(
                        eng,
                        h1_4,
                        h1_4,
                        ac3[:, 0][:, :, None].broadcast_to([P, 16, bb, W]),
                        ALU.add,
                    )
                    for hs, hd, cnt, d in levels:
                        he, ho = hview(hs, cnt, b0, b1)
                        hdv = hd[:].rearrange("p (j b u) -> p j b u", b=BATCH, u=W)[
                            :, :, b0:b1
                        ]
                        vv = svw(s_in, d, b0, b1)[:, None].broadcast_to([P, cnt, bb, W])
                        tt4(eng, hdv, ho, vv, ALU.mult)
                        tt4(eng, hdv, hdv, he, ALU.add)
                    # last level: h5 = h4_odd * v0; the final add lands straight
                    # in s_out (no clip needed: the result is a convex
                    # combination of sigmoids, always inside (0,1))
                    he, ho = hview(h4, 1, b0, b1)
                    h5v = h5[:].rearrange("p (b u) -> p b u", u=W)[:, b0:b1]
                    vv = svw(s_in, -2, b0, b1)
                    eng.tensor_tensor(h5v, ho[:, 0], vv, ALU.mult)
                    return he[:, 0], h5v

                def l5_add(eng, he, h5v, out_v, u0, u1):
                    eng.tensor_tensor(
                        out_v[:, :, u0:u1], h5v[:, :, u0:u1], he[:, :, u0:u1], ALU.add
                    )

                B_V = BATCH - GP_B
                chains = []
                step = B_V // V_SPLIT
                for v0 in range(0, B_V, step):
                    he_c, h5_c = tree(nc.vector, v0, min(v0 + step, B_V))
                    chains.append(
                        (nc.vector, he_c, h5_c, svw(s_out, 0, v0, min(v0 + step, B_V)))
                    )
                if GP_B:
                    he_g, h5_g = tree(nc.gpsimd, B_V, BATCH)
                    chains.append((nc.gpsimd, he_g, h5_g, svw(s_out, 0, B_V, BATCH)))
                # halo source columns (u 0,1 and 15,16) first, so the halo
                # DMAs overlap the bulk of the final add
                for eng, he_, h5_, ov in chains:
                    l5_add(eng, he_, h5_, ov, 0, 2)
                    l5_add(eng, he_, h5_, ov, W - 2, W)

                # halo refresh from neighboring partitions (b-major rows)
                r_out = s_out[:].rearrange("p (b w) -> p b w", w=WP)
                nc.sync.dma_start(
                    out=r_out[1:P, :, 0:2], in_=r_out[0 : P - 1, :, W : W + 2]
                )
                nc.sync.dma_start(
                    out=r_out[0 : P - 1, :, W + 2 : W + 4], in_=r_out[1:P, :, 2:4]
                )

                for eng, he_, h5_, ov in chains:
                    l5_add(eng, he_, h5_, ov, 2, W - 2)

            s_fin = s_a if LAYERS % 2 == 0 else s_b
            nc.sync.dma_start(
                out=out_d.ap().rearrange("p (b u) -> p b u", u=W), in_=sview(s_fin, 0)
            )

    nc.compile()
    return nc


def _shard_inputs(x, toggle_gates):
    """Host-side layout: chunked initial state + per-core gate windows."""
    s0f = np.zeros((BATCH, GRID), dtype=np.float32)
    s0f[:, ::STRIDE] = x
    tmat = _moebius_lhsT()
    in_maps = []
    p_idx = W * np.arange(P)[:, None] + np.arange(WP)[None, :] - 2  # (P, WP)
    for c in range(NCORES):
        a0 = CORE_N * c - HALO
        idx = (a0 + p_idx) % GRID
        s0_core = np.ascontiguousarray(
            s0f[:, idx].transpose(1, 0, 2).reshape(P, WP * BATCH)
        )
        gidx = (a0 + np.arange(SPAN_COLS)) % GRID
        twc_core = np.ascontiguousarray(toggle_gates[:, :, gidx])
        in_maps.append({"s0": s0_core, "twc": twc_core, "tmat": tmat})
    return in_maps


def _unshard(results):
    y = np.empty((BATCH, M), dtype=np.float32)
    ks = np.arange(CORE_N // STRIDE)  # 512 outputs per core
    off = HALO + STRIDE * ks  # position within the core's 2176-cell span
    for c in range(NCORES):
        dump = results[c]["out"].reshape(P, BATCH, W)
        y[:, (CORE_N // STRIDE) * c + ks] = dump[off // W, :, off % W].T
    return y


def kernel(x: np.ndarray, toggle_gates: np.ndarray) -> np.ndarray:
    if "nc" not in _CACHE:
        _CACHE["nc"] = _build_program()
    nc = _CACHE["nc"]
    in_maps = _shard_inputs(
        np.asarray(x, dtype=np.float32), np.asarray(toggle_gates, dtype=np.float32)
    )
    res = bass_utils.run_bass_kernel_spmd(nc, in_maps, core_ids=list(range(NCORES)))
    return _unshard(res.results)



# revision 18
# speedup vs baseline: 3.7303x; 3.7303x over previous
"""Trainium2 Bass kernel for nn_ASIC_44186623541335.

Soft-logic-gate cellular automaton: 8 layers over a 16384-cell ring; each cell
update is a 32-combo soft-boolean match over its 5-neighborhood, weighted by
sigmoid gates and clipped to [0,1].

Algorithm: the per-cell update  sum_c tw[c,n] * prod_i mu(bit_ci, v_i)  is the
multilinear extension of the gate vector tw[:,n] at the 5 neighbor values.
Per layer the gate vector is Moebius-transformed (a fixed 32x32 +/-1 matrix)
into monomial coefficients A_S[n]; the evaluation is a 31-node Horner tree.

v2 design (vs the fp32 baseline):
- The whole tree runs in fp16: DVE tensor_tensor gets the 2x_1P perf mode
  (2 elem/cycle/lane) when every operand AP has a 2-byte dtype, innermost
  step +-1 and 4-byte-aligned run starts.  All tiles therefore use even
  element strides (state rows 22, h/ac rows 18); the odd-offset neighbor
  views (d=+-1) read from a 1-cell-shifted copy of the state maintained by
  the Scalar engine.  fp16 keeps max rel err ~2e-3 (simulated), well under
  the 2e-2 gate.
- Halo refresh between layers is staged: DVE writes the 2 edge sums x 32
  batch straight from the Horner tail into a compact (cell,batch) tile, one
  partition-shifted DMA moves 128B/partition (127 descriptors instead of the
  4064 8-byte ones of a raw scattered halo DMA, which costs ~19us/layer on
  HW), ACT scatters back into the interleaved halo columns.  The next
  layer's level-1 op is split at UI=14 so its interior runs while the DMA
  flies; only a 3-column edge op waits for the scatter.
- Coefficient re-layout (sigma-major per partition) is done on the PE:
  17 matmuls with stride-17 lhsT slices of sigmoid(gates) put A directly
  into PSUM as [cell-partition, (u, sigma)]; one strided ACT copy writes the
  [p, sigma(stride 18), u] fp16 tile the tree reads.  This replaces the
  baseline's DRAM round-trip whose gather DMA had 4096 tiny descriptors.
- Layer 1 exploits the stride-4 embed sparsity: products of >=2 neighbors
  vanish unless the neighbors are 4 apart, so the full multilinear update
  collapses to 11 elementwise ops (exact, not an approximation).

Sharding: grid axis split across 8 cores (2048 cells each + 16-cell shrinking
halo, no inter-core communication). Per core the slice lives as 128 partitions
x 17 cells, free dim = (batch, cell).
"""

import numpy as np

import concourse.bacc as bacc
import concourse.mybir as mybir
from concourse import bass_utils
from concourse.tile import TileContext

GRID = 16384
LAYERS = 8
NPOS = 32
BATCH = 32
M = 4096
STRIDE = 4
NCORES = 8
CORE_N = GRID // NCORES  # 2048

P = 128           # partitions (grid chunks per core)
W = 17            # output cells per partition
WP = 22           # stored cells per partition (2-cell halo each side + 1 pad)
ACS = 18          # ac row stride per sigma (W padded to even)
HALO = 2 * LAYERS  # 16: core-level shrinking margin
SPAN_COLS = P * W  # 2176 cells covered per core

F32 = mybir.dt.float32
F16 = mybir.dt.float16
ALU = mybir.AluOpType
ACT_SIGMOID = mybir.ActivationFunctionType.Sigmoid

# batch columns handled by GPSIMD (rest on VectorE); 0 disables the split
GP_B = 0

_CACHE = {}


def _moebius_lhsT():
    """rhs[c, s] = T[s, c]; A_s = sum_c T[s,c] * tw_c  (multilinear coeffs).

    T[s, c] = (-1)^popcount(s & ~c) if (c & ~s) == 0 else 0.
    sigma bit b corresponds to neighbor shift d = 2 - b.
    """
    t = np.zeros((NPOS, NPOS), dtype=np.float32)
    for s in range(NPOS):
        for c in range(NPOS):
            if c & ~s & 31:
                continue
            t[s, c] = (-1.0) ** bin(s & ~c & 31).count("1")
    return np.ascontiguousarray(t.T)


def _build_program(nrep=None):
    """nrep: timing-only variant -- wraps the 8-layer body in a hardware
    loop executed nrep times (results are then meaningless; used to
    measure per-layer cost from wall-clock deltas)."""
    import contextlib

    nc = bacc.Bacc("TRN2", target_bir_lowering=False, debug=False)
    s0_d = nc.dram_tensor("s0", [P, BATCH * WP], F16, kind="ExternalInput")
    twc_d = nc.dram_tensor(
        "twc", [LAYERS, NPOS, SPAN_COLS], F32, kind="ExternalInput"
    )
    tm_d = nc.dram_tensor("tmat", [NPOS, NPOS], F32, kind="ExternalInput")
    out_d = nc.dram_tensor("out", [P, W * BATCH], F16, kind="ExternalOutput")

    HB = 16 * BATCH * ACS  # h1 allocation unit

    with TileContext(nc) as tc:
        with (
            tc.tile_pool(name="state", bufs=1) as sp,
            tc.tile_pool(name="coef", bufs=2) as cp,
            tc.tile_pool(name="psum", bufs=2, space="PSUM") as pp,
        ):
            s_a = sp.tile([P, BATCH * WP], F16, tag="s_a")
            s_b = sp.tile([P, BATCH * WP], F16, tag="s_b")
            s1 = sp.tile([P, BATCH * WP], F16, tag="s1")
            h1 = sp.tile([P, 16 * BATCH * ACS], F16, tag="h1")
            h2 = sp.tile([P, 8 * BATCH * ACS], F16, tag="h2")
            h3 = sp.tile([P, 4 * BATCH * ACS], F16, tag="h3")
            h4 = sp.tile([P, 2 * BATCH * ACS], F16, tag="h4")
            h5 = sp.tile([P, BATCH * ACS], F16, tag="h5")
            tmat = sp.tile([NPOS, NPOS], F32, tag="tmat")
            stg = sp.tile([P, 4 * BATCH], F16, tag="stg")
            sth = sp.tile([P, 4 * BATCH], F16, tag="sth")
            sc = sp.tile([P, BATCH * W], F16, tag="sc")

            # edge-partition halo columns are never refreshed; zero-fill once
            # so reads stay finite (values only feed the discarded margin)
            nc.gpsimd.memset(s_a[:], 0.0)
            nc.gpsimd.memset(s_b[:], 0.0)
            nc.gpsimd.memset(s1[:], 0.0)
            nc.gpsimd.memset(sth[:], 0.0)

            nc.sync.dma_start(out=s_a[:], in_=s0_d.ap())
            nc.sync.dma_start(out=tmat[:], in_=tm_d.ap())

            def sr(s):
                return s[:].rearrange("p (b w) -> p b w", w=WP)

            def sview(s, d):
                # state view shifted by d cells (d even): (P, BATCH, W)
                return sr(s)[:, :, 2 + d : 2 + d + W]

            def s1view(d):
                # shifted-copy view for odd d: s1[c] = s[c+1]
                return sr(s1)[:, :, 1 + d : 1 + d + W]

            def hv(hs, cnt):
                # h tile as (P, cnt, BATCH, W) with ACS row stride
                return hs[:].rearrange(
                    "p (j b u) -> p j b u", b=BATCH, u=ACS
                )[:, 0:cnt, :, 0:W]

            # initial shifted copy for layer-1 odd views
            nc.scalar.copy(sr(s1)[:, :, 0:WP - 1], sr(s_a)[:, :, 1:WP])

            def emit_coef(layer):
                """Coefficient pipeline (PE path, no DRAM bounce): returns the
                fp16 [p, sigma(stride ACS), u] coefficient view for `layer`."""
                twr = cp.tile([NPOS, SPAN_COLS], F32, tag="twr")
                tws = cp.tile([NPOS, SPAN_COLS], F32, tag="tws")
                ac = cp.tile([P, NPOS * ACS], F16, tag="ac")
                pc = pp.tile([P, W * NPOS], F32, tag="pc")

                nc.sync.dma_start(out=twr[:], in_=twc_d.ap()[layer])
                nc.scalar.activation(tws[:], twr[:], ACT_SIGMOID)
                tws_m = tws[:].rearrange("c (m u) -> c m u", u=W)
                pc_v = pc[:].rearrange("p (u s) -> p u s", s=NPOS)
                for u in range(W):
                    nc.tensor.matmul(
                        pc_v[:, u], tws_m[:, :, u], tmat[:],
                        start=True, stop=True,
                    )
                # transpose (u, sigma) -> (sigma-major stride ACS) + fp16 cast
                acv = ac[:].rearrange("p (s u) -> p s u", u=ACS)
                nc.scalar.copy(
                    acv[:, :, 0:W],
                    pc_v.rearrange("p u s -> p s u"),
                )
                return acv

            loop_cm = tc.For_i(0, nrep, 1) if nrep else contextlib.nullcontext()
            acv_cur = emit_coef(0)
            with loop_cm:
              for layer in range(LAYERS):
                s_in = s_a if layer % 2 == 0 else s_b
                s_out = s_b if layer % 2 == 0 else s_a

                def vin(d, b0=0, b1=BATCH):
                    v = sview(s_in, d) if d % 2 == 0 else s1view(d)
                    return v[:, b0:b1]

                acv = acv_cur
                # prefetch next layer's coefficients while this tree runs
                # (in the timing loop, layer 7 prefetches layer 0 of the next
                # iteration -- wrong values, representative cost)
                if layer < LAYERS - 1:
                    acv_next = emit_coef(layer + 1)
                elif nrep:
                    acv_next = emit_coef(0)
                else:
                    acv_next = None
                acv_cur = acv_next

                def acs(sigma, b0=0, b1=BATCH):
                    # A_sigma broadcast over batch slice: (P, b, W)
                    return acv[:, sigma, None, 0:W].broadcast_to(
                        [P, b1 - b0, W]
                    )

                def tree(eng, b0, b1):
                    """31-node Horner tree on batch slice [b0, b1)."""
                    bb = b1 - b0
                    ac2 = acv[:, :, None, 0:W]  # (P, 32, 1, W)

                    def acj(parity, cnt):
                        return ac2[:, parity::2][:, 0:cnt].broadcast_to(
                            [P, cnt, bb, W]
                        )

                    h1v = hv(h1, 16)[:, :, b0:b1]
                    v = vin(2, b0, b1)[:, None].broadcast_to([P, 16, bb, W])
                    # split: interior cols don't read the right halo, so this
                    # part starts before the halo DMA lands (UI = even split)
                    UI = 14
                    eng.tensor_tensor(
                        h1v[:, :, :, 0:UI], acj(1, 16)[:, :, :, 0:UI],
                        v[:, :, :, 0:UI], ALU.mult,
                    )
                    eng.tensor_tensor(
                        h1v[:, :, :, UI:W], acj(1, 16)[:, :, :, UI:W],
                        v[:, :, :, UI:W], ALU.mult,
                    )
                    eng.tensor_tensor(h1v, h1v, acj(0, 16), ALU.add)
                    for hs, hd, cnt, d in (
                        (h1, h2, 8, 1), (h2, h3, 4, 0), (h3, h4, 2, -1)
                    ):
                        r = hv(hs, 2 * cnt)[:, :, b0:b1]
                        he, ho = r[:, 0::2], r[:, 1::2]
                        hdv = hv(hd, cnt)[:, :, b0:b1]
                        v = vin(d, b0, b1)[:, None].broadcast_to(
                            [P, cnt, bb, W]
                        )
                        eng.tensor_tensor(hdv, ho, v, ALU.mult)
                        eng.tensor_tensor(hdv, hdv, he, ALU.add)
                    r = hv(h4, 2)[:, :, b0:b1]
                    he, ho = r[:, 0], r[:, 1]
                    h5v = hv(h5, 1)[:, 0, b0:b1]
                    eng.tensor_tensor(h5v, ho, vin(-2, b0, b1), ALU.mult)
                    return he, h5v

                def tree_l0(eng, b0, b1):
                    """Layer-1 sparse-exact update: state has x at stride 4,
                    so out = A_0 + sum_d A_{1<<(2-d)} v(d)
                               + A_17 v(-2) v(+2)   (11 ops, exact)."""
                    q = hv(h4, 1)[:, 0, b0:b1]
                    t = hv(h5, 1)[:, 0, b0:b1]
                    # q = v(-2) * (A16 + A17*v(+2)) + A_0
                    eng.tensor_tensor(q, acs(17, b0, b1), vin(2, b0, b1), ALU.mult)
                    eng.tensor_tensor(q, q, acs(16, b0, b1), ALU.add)
                    eng.tensor_tensor(q, q, vin(-2, b0, b1), ALU.mult)
                    eng.tensor_tensor(q, q, acs(0, b0, b1), ALU.add)
                    for d, sigma in ((2, 1), (1, 2), (0, 4)):
                        eng.tensor_tensor(t, acs(sigma, b0, b1), vin(d, b0, b1), ALU.mult)
                        eng.tensor_tensor(q, q, t, ALU.add)
                    # last term stays separate; the final adds fold it in
                    eng.tensor_tensor(t, acs(8, b0, b1), vin(-1, b0, b1), ALU.mult)
                    return t, q

                B_V = BATCH - GP_B
                chains = []
                if layer == 0:
                    he_c, h5_c = tree_l0(nc.vector, 0, B_V)
                    chains.append((nc.vector, he_c, h5_c, sview(s_out, 0)[:, 0:B_V]))
                    if GP_B:
                        he_g, h5_g = tree_l0(nc.gpsimd, B_V, BATCH)
                        chains.append((nc.gpsimd, he_g, h5_g, sview(s_out, 0)[:, B_V:BATCH]))
                else:
                    he_c, h5_c = tree(nc.vector, 0, B_V)
                    chains.append((nc.vector, he_c, h5_c, sview(s_out, 0)[:, 0:B_V]))
                    if GP_B:
                        he_g, h5_g = tree(nc.gpsimd, B_V, BATCH)
                        chains.append((nc.gpsimd, he_g, h5_g, sview(s_out, 0)[:, B_V:BATCH]))

                last = layer == LAYERS - 1
                if not last:
                    # ---- staged halo refresh (right side first: the next
                    # layer's level-1 edge op reads the right halo) ----
                    # DVE computes the edge sums straight from h5/he into the
                    # compact staging rows -- the halo DMAs fly before the
                    # final add even starts.
                    r_out = sr(s_out)
                    sg = stg[:].rearrange("p (u b) -> p u b", b=BATCH)
                    sh = sth[:].rearrange("p (u b) -> p u b", b=BATCH)
                    for eng, he_, h5_, ov in chains:
                        eng.tensor_tensor(
                            sg[:, 2:4].rearrange("p u b -> p b u"),
                            h5_[:, :, 0:2], he_[:, :, 0:2], ALU.add,
                        )
                    nc.scalar.dma_start(out=sh[0:P - 1, 2:4], in_=sg[1:P, 2:4])
                    for eng, he_, h5_, ov in chains:
                        eng.tensor_tensor(
                            sg[:, 0:2].rearrange("p u b -> p b u"),
                            h5_[:, :, W - 2:W], he_[:, :, W - 2:W], ALU.add,
                        )
                    nc.sync.dma_start(out=sh[1:P, 0:2], in_=sg[0:P - 1, 0:2])

                # single full-width final add (overlaps the halo DMAs)
                for eng, he_, h5_, ov in chains:
                    eng.tensor_tensor(ov, h5_, he_, ALU.add)

                if not last:
                    # right scatter first (level-1 of the next layer reads it),
                    # then the s1 column that depends on it
                    nc.scalar.copy(
                        r_out[:, :, W + 2:W + 4].rearrange("p b u -> p u b"),
                        sh[:, 2:4],
                    )
                    nc.scalar.copy(
                        sr(s1)[:, :, W + 1:W + 2], sr(s_out)[:, :, W + 2:W + 3]
                    )
                    # bulk of the shifted copy: cols 1..17 <- s cols 2..18
                    # (hidden under the next layer's level-1 op)
                    nc.scalar.copy(sr(s1)[:, :, 1:W + 1], sr(s_out)[:, :, 2:W + 2])
                    # left scatter + its s1 column (needed from level 4 on)
                    nc.scalar.copy(
                        r_out[:, :, 0:2].rearrange("p b u -> p u b"), sh[:, 0:2]
                    )
                    nc.scalar.copy(sr(s1)[:, :, 0:1], sr(s_out)[:, :, 1:2])

            s_fin = s_a if LAYERS % 2 == 0 else s_b
            # compact (no pad stride) staging so the output DMA has
            # 1088B-contiguous rows instead of 34B ones
            nc.vector.tensor_copy(
                sc[:].rearrange("p (b u) -> p b u", u=W), sview(s_fin, 0)
            )
            nc.sync.dma_start(out=out_d.ap(), in_=sc[:])

    nc.compile()
    return nc


def _shard_inputs(x, toggle_gates):
    """Host-side layout: chunked initial state + per-core gate windows."""
    s0f = np.zeros((BATCH, GRID), dtype=np.float32)
    s0f[:, ::STRIDE] = x
    tmat = _moebius_lhsT()
    in_maps = []
    p_idx = W * np.arange(P)[:, None] + np.arange(WP)[None, :] - 2  # (P, WP)
    for c in range(NCORES):
        a0 = CORE_N * c - HALO
        idx = (a0 + p_idx) % GRID
        s0_core = np.ascontiguousarray(
            s0f[:, idx].transpose(1, 0, 2).reshape(P, BATCH * WP)
        ).astype(np.float16)
        gidx = (a0 + np.arange(SPAN_COLS)) % GRID
        twc_core = np.ascontiguousarray(toggle_gates[:, :, gidx])
        in_maps.append({"s0": s0_core, "twc": twc_core, "tmat": tmat})
    return in_maps


def _unshard(results):
    y = np.empty((BATCH, M), dtype=np.float32)
    ks = np.arange(CORE_N // STRIDE)  # 512 outputs per core
    off = HALO + STRIDE * ks  # position within the core's 2176-cell span
    for c in range(NCORES):
        dump = results[c]["out"].astype(np.float32).reshape(P, BATCH, W)
        y[:, (CORE_N // STRIDE) * c + ks] = dump[off // W, :, off % W].T
    return y


def kernel(x: np.ndarray, toggle_gates: np.ndarray) -> np.ndarray:
    if "nc" not in _CACHE:
        _CACHE["nc"] = _build_program()
    nc = _CACHE["nc"]
    in_maps = _shard_inputs(
        np.asarray(x, dtype=np.float32), np.asarray(toggle_gates, dtype=np.float32)
    )
    res = bass_utils.run_bass_kernel_spmd(nc, in_maps, core_ids=list(range(NCORES)))
    return _unshard(res.results)
